# revision 31
# baseline (speedup 1.0000x reference)
"""Causal self-attention with RoPE on 8 Trainium2 NeuronCores.

Problem: B=2, S=2048, H=16 heads, D=128, HID=2048, fp32.
  qkv = x @ w_qkv.T ; RoPE(q, k) ; causal softmax(q k^T / sqrt(D)) @ v ; out @ w_o.T

Sharding (hardcoded): core c in 0..7 handles batch b = c // 4 and head group
g = c % 4 (heads 4g..4g+4). Each core computes a partial (S, HID) output
contracted over its 512 hidden dims of the o-projection; the host sums the 4
partials per batch.

Dataflow: the q/k projections are computed directly transposed (q^T/k^T tiles
[d=128 partitions, s free]; host pre-transposes x and weights so the
contraction dim h lands on partitions). RoPE is applied in place per
(head, s-half) as soon as that half's q/k land, hiding under the remaining
projection matmuls. V is projected in natural [s, d] layout (all 4 heads
fused per s-tile so the moving dim stays 512). Scores are computed transposed
(scores^T[sj, si]), softmax runs without max-subtraction (scores are O(5);
exp is safe in fp32), the denominator is a ones-vector matmul over
partitions, causal masking is an additive triangle on the diagonal 128x128
blocks plus column-restricted matmuls below the diagonal, and attention
output comes out as out^T[d, si] — exactly the layout the o-projection needs
as lhsT.
"""

import os

import numpy as np

import concourse.bacc as bacc
import concourse.tile as tile
from concourse import mybir
from concourse.bass_utils import run_bass_kernel_spmd

B, S, H, D = 2, 2048, 16, 128
HID = H * D
THETA = 10000.0
SCALE = 1.0 / float(np.sqrt(D))
NH = 4                 # heads per core
NC = 8                 # cores
NKC = HID // 128       # contraction chunks for qkv projection
SB = 512               # matmul moving free dim
NSB = S // SB          # si blocks
SH = S // 2            # s-half
F32 = mybir.dt.float32

# "fp32r" -> TF32-class matmuls at ~2x the fp32 rate (rel err ~2e-4)
# "fp32"  -> full fp32 matmuls (rel err ~2e-6)
MM_MODE = os.environ.get("BASS_MM_MODE", "fp32r")
MMDT = mybir.dt.float32r if MM_MODE == "fp32r" else mybir.dt.float32

LAST_RESULT = None  # BassKernelResults of the most recent run (for test harness)


def _build_nc():
    nc = bacc.Bacc("TRN2", target_bir_lowering=False, debug=False, num_devices=NC)

    xT = nc.dram_tensor("xT", [HID, S], F32, kind="ExternalInput")
    wqkvT = nc.dram_tensor("wqkvT", [HID, 3 * NH * 128], F32, kind="ExternalInput")
    woT = nc.dram_tensor("woT", [NH * 128, HID], F32, kind="ExternalInput")
    cosT = nc.dram_tensor("cosT", [D, S], F32, kind="ExternalInput")
    sinST = nc.dram_tensor("sinST", [D, S], F32, kind="ExternalInput")
    maskadd = nc.dram_tensor("maskadd", [128, 128], F32, kind="ExternalInput")
    out = nc.dram_tensor("out", [S, HID], F32, kind="ExternalOutput")

    with tile.TileContext(nc) as tc:
        with tc.tile_pool(name="pmisc", bufs=1) as pmisc, \
             tc.tile_pool(name="pqk", bufs=1) as pqk, \
             tc.tile_pool(name="pvn", bufs=1) as pvn:
            ones_t = pmisc.tile([128, 1], MMDT, name="ones")
            if MMDT == F32:
                nc.vector.memset(ones_t, 1.0)
            else:
                ones_f32 = pmisc.tile([128, 1], F32, name="ones_f32")
                nc.vector.memset(ones_f32, 1.0)
                nc.vector.tensor_copy(ones_t, ones_f32)
            tri_t = pmisc.tile([128, 128], F32, name="tri")
            nc.scalar.dma_start(out=tri_t, in_=maskadd[:, :])

            qT = [pqk.tile([128, S], MMDT, name=f"qT_{h}") for h in range(NH)]
            kT = [pqk.tile([128, S], MMDT, name=f"kT_{h}") for h in range(NH)]
            # natural-layout v, all heads fused: vn[g][sp, j, h*128+d] covers
            # s-chunks 4g+j
            vn = [pvn.tile([128, 4, NH * 128], MMDT, name=f"vn_{g}")
                  for g in range(4)]
            # attention output aliases qT: q columns for an si-block are dead
            # once that block's scores are done, and the normalized output is
            # written only after that point.
            outT = qT

            # ---- Phase A: q/k/v projection + RoPE, per s-half ----
            with tc.tile_pool(name="px", bufs=1) as px, \
                 tc.tile_pool(name="ptrig", bufs=1) as ptrig, \
                 tc.tile_pool(name="psh", bufs=1) as psh:
                for half in range(2):
                    s0 = half * SH
                    xh = []
                    for kc in range(NKC):
                        xt = px.tile([128, SH], MMDT, name=f"xh_{kc}")
                        nc.sync.dma_start(
                            out=xt, in_=xT[kc * 128:(kc + 1) * 128,
                                           s0:s0 + SH].bitcast(MMDT))
                        xh.append(xt)
                    cos_t = ptrig.tile([D, SH], F32, name="cosT")
                    sin_t = ptrig.tile([D, SH], F32, name="sinST")
                    nc.scalar.dma_start(out=cos_t, in_=cosT[:, s0:s0 + SH])
                    nc.scalar.dma_start(out=sin_t, in_=sinST[:, s0:s0 + SH])

                    # q/k projection, transposed output [d, s]
                    with tc.tile_pool(name="pw", bufs=4) as pw, \
                         tc.tile_pool(name="ppa", bufs=8, space="PSUM") as ppa:
                        for h in range(NH):
                            for kind, dst in ((0, qT[h]), (1, kT[h])):
                                ot = kind * NH + h
                                wt = pw.tile([128, NKC, 128], MMDT, name="wt")
                                nc.scalar.dma_start(
                                    out=wt,
                                    in_=wqkvT[:, ot * 128:(ot + 1) * 128]
                                    .rearrange("(kc p) o -> p kc o", p=128)
                                    .bitcast(MMDT))
                                for sb_i in range(SH // SB):
                                    ps_t = ppa.tile([128, SB], F32, name="qkps")
                                    for kc in range(NKC):
                                        nc.tensor.matmul(
                                            ps_t, wt[:, kc, :],
                                            xh[kc][:, sb_i * SB:(sb_i + 1) * SB],
                                            start=(kc == 0), stop=(kc == NKC - 1))
                                    lo = s0 + sb_i * SB
                                    nc.any.tensor_copy(dst[:, lo:lo + SB], ps_t)
                            # RoPE for this head's half, in place (hides
                            # under the remaining projection matmuls)
                            for t in (qT[h], kT[h]):
                                sh_t = psh.tile([128, SH], MMDT, name="shuf")
                                nc.gpsimd.dma_start(out=sh_t[0:64, :],
                                                    in_=t[64:128, s0:s0 + SH])
                                nc.gpsimd.dma_start(out=sh_t[64:128, :],
                                                    in_=t[0:64, s0:s0 + SH])
                                nc.vector.tensor_mul(sh_t, sh_t, sin_t)
                                nc.vector.tensor_mul(t[:, s0:s0 + SH],
                                                     t[:, s0:s0 + SH], cos_t)
                                nc.vector.tensor_add(t[:, s0:s0 + SH],
                                                     t[:, s0:s0 + SH], sh_t)

                    # v projection, natural layout [s, 4 heads x d]
                    with tc.tile_pool(name="pwv", bufs=4) as pwv, \
                         tc.tile_pool(name="pvp", bufs=1, space="PSUM") as pvp:
                        vps = [pvp.tile([128, NH * 128], F32, name=f"vps_{st}")
                               for st in range(8)]
                        for kc in range(NKC):
                            wv = pwv.tile([128, NH * 128], MMDT, name="wv")
                            nc.sync.dma_start(
                                out=wv,
                                in_=wqkvT[kc * 128:(kc + 1) * 128,
                                          2 * NH * 128:].bitcast(MMDT))
                            for st in range(8):
                                nc.tensor.matmul(
                                    vps[st],
                                    xh[kc][:, st * 128:(st + 1) * 128],
                                    wv,
                                    start=(kc == 0), stop=(kc == NKC - 1))
                        for st in range(8):
                            sg = half * 8 + st   # global s-chunk
                            nc.any.tensor_copy(vn[sg // 4][:, sg % 4, :], vps[st])

            # ---- Phase B: attention per head ----
            with tc.tile_pool(name="pwo", bufs=1) as pwo:
              wo = []
              for h in range(NH):
                  wt = pwo.tile([128, HID], MMDT, name=f"wo_{h}")
                  nc.scalar.dma_start(
                      out=wt, in_=woT[h * 128:(h + 1) * 128, :].bitcast(MMDT))
                  wo.append(wt)
              with tc.tile_pool(name="pexp", bufs=4) as pexp, \
                   tc.tile_pool(name="prr", bufs=3) as prr, \
                   tc.tile_pool(name="psc", bufs=2, space="PSUM") as psc, \
                   tc.tile_pool(name="plp", bufs=1, space="PSUM") as plp, \
                   tc.tile_pool(name="pop", bufs=3, space="PSUM") as pop:
                  for h in range(NH):
                      for sib in range(NSB):
                          si0 = sib * SB
                          nch = 4 * (sib + 1)      # sj chunks (128 each)
                          l_ps = plp.tile([1, SB], F32, name="lps")
                          o_ps = pop.tile([128, SB], F32, name="ops")
                          for cp in range(nch // 2):
                              s_ps = psc.tile([128, 2, SB], F32, name="sps")
                              e_t = pexp.tile([128, 2, SB], MMDT, name="exp")
                              los = []
                              for j in range(2):
                                  cj = cp * 2 + j
                                  dg = cj - (nch - 4)   # diagonal offset
                                  lo = dg * 128 if dg > 0 else 0
                                  los.append((cj, dg, lo))
                                  nc.tensor.matmul(
                                      s_ps[:, j, lo:],
                                      kT[h][:, cj * 128:(cj + 1) * 128],
                                      qT[h][:, si0 + lo:si0 + SB],
                                      start=True, stop=True)
                                  if dg >= 0:
                                      nc.vector.tensor_add(
                                          s_ps[:, j, lo:lo + 128],
                                          s_ps[:, j, lo:lo + 128], tri_t)
                              if los[0][2] == 0 and los[1][2] == 0:
                                  nc.scalar.activation(
                                      out=e_t, in_=s_ps,
                                      func=mybir.ActivationFunctionType.Exp,
                                      scale=SCALE)
                              else:
                                  for j in range(2):
                                      lo = los[j][2]
                                      nc.scalar.activation(
                                          out=e_t[:, j, lo:], in_=s_ps[:, j, lo:],
                                          func=mybir.ActivationFunctionType.Exp,
                                          scale=SCALE)
                              for j in range(2):
                                  cj, dg, lo = los[j]
                                  nc.tensor.matmul(
                                      l_ps[:, lo:], ones_t, e_t[:, j, lo:],
                                      start=(cj == 0), stop=(cj == nch - 1))
                                  nc.tensor.matmul(
                                      o_ps[:, lo:],
                                      vn[cj // 4][:, cj % 4,
                                                  h * 128:(h + 1) * 128],
                                      e_t[:, j, lo:],
                                      start=(cj == 0), stop=(cj == nch - 1))
                          # 1/l as exp(-ln(l)) on ACT: both functions live
                          # in one table set, and this is ~2x faster than the
                          # single-lane DVE iterative divide
                          lnl = prr.tile([1, SB], F32, name="lnl")
                          nc.scalar.activation(
                              out=lnl, in_=l_ps,
                              func=mybir.ActivationFunctionType.Ln)
                          recip = prr.tile([1, SB], F32, name="recip")
                          nc.scalar.activation(
                              out=recip, in_=lnl,
                              func=mybir.ActivationFunctionType.Exp,
                              scale=-1.0)
                          rb = prr.tile([128, SB], F32, name="rb")
                          nc.gpsimd.partition_broadcast(rb, recip)
                          nc.vector.tensor_mul(
                              outT[h][:, si0:si0 + SB], o_ps, rb)

              # ---- Phase C: partial o-projection ----
              with tc.tile_pool(name="pfin", bufs=5) as pfin, \
                   tc.tile_pool(name="pfp", bufs=4, space="PSUM") as pfp:
                  for st in range(S // 128):
                      fin = pfin.tile([128, HID], F32, name="fin")
                      for ob in range(HID // SB):
                          fps = pfp.tile([128, SB], F32, name="fps")
                          for h in range(NH):
                              nc.tensor.matmul(
                                  fps,
                                  outT[h][:, st * 128:(st + 1) * 128],
                                  wo[h][:, ob * SB:(ob + 1) * SB],
                                  start=(h == 0), stop=(h == NH - 1))
                          nc.any.tensor_copy(fin[:, ob * SB:(ob + 1) * SB], fps)
                      nc.sync.dma_start(
                          out=out[st * 128:(st + 1) * 128, :], in_=fin)

    # Force exp and ln onto the single `natural_log_exp_and_others` ACT
    # table set: with the default map the table-load pass alternates between
    # the exp-only and ln-only sets (~33 reloads x 2.7us on ScalarE). Blank
    # the single-function sets (positions preserved, so set ids stay valid)
    # so both functions resolve to the combined set -> one load.
    import concourse.bacc as _bacc_mod
    import concourse.hw_specs as _hw_specs
    _orig_tables = _hw_specs.get_activation_tables

    def _patched_tables(arch):
        t = dict(_orig_tables(arch))
        for name in ("exp_and_others", "exp_and_friends", "natural_log"):
            if name in t:
                t[name] = set()
        return t

    _bacc_mod.get_activation_tables = _patched_tables
    try:
        nc.compile()
    finally:
        _bacc_mod.get_activation_tables = _orig_tables
    return nc


_NC_CACHE = None


def _get_nc():
    global _NC_CACHE
    if _NC_CACHE is None:
        _NC_CACHE = _build_nc()
    return _NC_CACHE


def _host_inputs(x, w_qkv, w_o):
    """Per-core input maps (sharding + layout prep on host)."""
    inv_freq = 1.0 / (THETA ** (np.arange(0, D, 2, dtype=np.float64) / D))
    pos = np.arange(S, dtype=np.float64)
    freqs = pos[:, None] * inv_freq[None, :]          # (S, D/2)
    emb = np.concatenate([freqs, freqs], axis=-1)     # (S, D)
    cosT = np.ascontiguousarray(np.cos(emb).T.astype(np.float32))   # (D, S)
    sign = np.concatenate([-np.ones(D // 2), np.ones(D // 2)])
    sinST = np.ascontiguousarray((sign[None, :] * np.sin(emb)).T
                                 .astype(np.float32))               # (D, S)
    # additive causal triangle for a diagonal 128x128 block of scores^T:
    # keep (add 0) when sj_local <= si_local, else -1e30
    p = np.arange(128)[:, None]
    f = np.arange(128)[None, :]
    maskadd = np.where(p <= f, 0.0, -1e30).astype(np.float32)       # (128, 128)

    xTb = [np.ascontiguousarray(x[b].T) for b in range(B)]          # (HID, S)
    in_maps = []
    for c in range(NC):
        b, g = c // 4, c % 4
        rows = slice(g * NH * D, (g + 1) * NH * D)
        wq = w_qkv[0 * HID:1 * HID][rows]
        wk = w_qkv[1 * HID:2 * HID][rows]
        wv = w_qkv[2 * HID:3 * HID][rows]
        wqkvT = np.ascontiguousarray(
            np.concatenate([wq, wk, wv], axis=0).T)                 # (HID, 1536)
        woT = np.ascontiguousarray(w_o[:, rows].T)                  # (512, HID)
        in_maps.append({
            "xT": xTb[b], "wqkvT": wqkvT, "woT": woT,
            "cosT": cosT, "sinST": sinST, "maskadd": maskadd,
        })
    return in_maps


def kernel(x, w_qkv, w_o):
    global LAST_RESULT
    x = np.asarray(x, dtype=np.float32)
    w_qkv = np.asarray(w_qkv, dtype=np.float32)
    w_o = np.asarray(w_o, dtype=np.float32)

    nc = _get_nc()
    in_maps = _host_inputs(x, w_qkv, w_o)
    trace = bool(int(os.environ.get("BASS_KERNEL_TRACE", "0")))
    last_exc = None
    for _attempt in range(3):
        try:
            res = run_bass_kernel_spmd(
                nc, in_maps, core_ids=list(range(NC)),
                trace=trace, trace_cores=list(range(NC)) if trace else None)
            break
        except Exception as e:  # transient NRT device errors: retry
            last_exc = e
    else:
        raise last_exc
    LAST_RESULT = res

    out = np.empty((B, S, HID), dtype=np.float32)
    for b in range(B):
        acc = np.zeros((S, HID), dtype=np.float64)
        for g in range(4):
            acc += res.results[b * 4 + g]["out"]
        out[b] = acc.astype(np.float32)
    return out



# revision 32
# speedup vs baseline: 1.0074x; 1.0074x over previous
"""Causal self-attention with RoPE on 8 Trainium2 NeuronCores.

Problem: B=2, S=2048, H=16 heads, D=128, HID=2048, fp32.
  qkv = x @ w_qkv.T ; RoPE(q, k) ; causal softmax(q k^T / sqrt(D)) @ v ; out @ w_o.T

Sharding (hardcoded): core c in 0..7 handles batch b = c // 4 and head group
g = c % 4 (heads 4g..4g+4). Each core computes a partial (S, HID) output
contracted over its 512 hidden dims of the o-projection; the host sums the 4
partials per batch.

Dataflow: the q/k projections are computed directly transposed (q^T/k^T tiles
[d=128 partitions, s free]; host pre-transposes x and weights so the
contraction dim h lands on partitions). RoPE is applied in place per
(head, s-half) as soon as that half's q/k land, hiding under the remaining
projection matmuls. V is projected in natural [s, d] layout (all 4 heads
fused per s-tile so the moving dim stays 512). Scores are computed transposed
(scores^T[sj, si]), softmax runs without max-subtraction (scores are O(5);
exp is safe in fp32), the denominator is a ones-vector matmul over
partitions, causal masking is an additive triangle on the diagonal 128x128
blocks plus column-restricted matmuls below the diagonal, and attention
output comes out as out^T[d, si] — exactly the layout the o-projection needs
as lhsT.
"""

import os

import numpy as np

import concourse.bacc as bacc
import concourse.tile as tile
from concourse import mybir
from concourse.bass_utils import run_bass_kernel_spmd

B, S, H, D = 2, 2048, 16, 128
HID = H * D
THETA = 10000.0
SCALE = 1.0 / float(np.sqrt(D))
NH = 4                 # heads per core
NC = 8                 # cores
NKC = HID // 128       # contraction chunks for qkv projection
SB = 512               # matmul moving free dim
NSB = S // SB          # si blocks
SH = S // 2            # s-half
F32 = mybir.dt.float32

# "fp32r" -> TF32-class matmuls at ~2x the fp32 rate (rel err ~2e-4)
# "fp32"  -> full fp32 matmuls (rel err ~2e-6)
MM_MODE = os.environ.get("BASS_MM_MODE", "fp32r")
MMDT = mybir.dt.float32r if MM_MODE == "fp32r" else mybir.dt.float32

LAST_RESULT = None  # BassKernelResults of the most recent run (for test harness)


def _build_nc():
    nc = bacc.Bacc("TRN2", target_bir_lowering=False, debug=False, num_devices=NC)

    xT = nc.dram_tensor("xT", [HID, S], F32, kind="ExternalInput")
    wqkvT = nc.dram_tensor("wqkvT", [HID, 3 * NH * 128], F32, kind="ExternalInput")
    woT = nc.dram_tensor("woT", [NH * 128, HID], F32, kind="ExternalInput")
    cosT = nc.dram_tensor("cosT", [D, S], F32, kind="ExternalInput")
    sinST = nc.dram_tensor("sinST", [D, S], F32, kind="ExternalInput")
    maskadd = nc.dram_tensor("maskadd", [128, 128], F32, kind="ExternalInput")
    out = nc.dram_tensor("out", [S, HID], F32, kind="ExternalOutput")

    with tile.TileContext(nc) as tc:
        with tc.tile_pool(name="pmisc", bufs=1) as pmisc, \
             tc.tile_pool(name="pqk", bufs=1) as pqk, \
             tc.tile_pool(name="pvn", bufs=1) as pvn:
            ones_t = pmisc.tile([128, 1], MMDT, name="ones")
            if MMDT == F32:
                nc.vector.memset(ones_t, 1.0)
            else:
                ones_f32 = pmisc.tile([128, 1], F32, name="ones_f32")
                nc.vector.memset(ones_f32, 1.0)
                nc.vector.tensor_copy(ones_t, ones_f32)
            tri_t = pmisc.tile([128, 128], F32, name="tri")
            nc.scalar.dma_start(out=tri_t, in_=maskadd[:, :])

            qT = [pqk.tile([128, S], MMDT, name=f"qT_{h}") for h in range(NH)]
            kT = [pqk.tile([128, S], MMDT, name=f"kT_{h}") for h in range(NH)]
            # natural-layout v, all heads fused: vn[g][sp, j, h*128+d] covers
            # s-chunks 4g+j
            vn = [pvn.tile([128, 4, NH * 128], MMDT, name=f"vn_{g}")
                  for g in range(4)]
            # attention output aliases qT: q columns for an si-block are dead
            # once that block's scores are done, and the normalized output is
            # written only after that point.
            outT = qT

            # ---- Phase A: q/k/v projection + RoPE, per s-half ----
            with tc.tile_pool(name="px", bufs=1) as px, \
                 tc.tile_pool(name="ptrig", bufs=1) as ptrig, \
                 tc.tile_pool(name="psh", bufs=1) as psh:
                for half in range(2):
                    s0 = half * SH
                    xh = []
                    for kc in range(NKC):
                        xt = px.tile([128, SH], MMDT, name=f"xh_{kc}")
                        nc.sync.dma_start(
                            out=xt, in_=xT[kc * 128:(kc + 1) * 128,
                                           s0:s0 + SH].bitcast(MMDT))
                        xh.append(xt)
                    cos_t = ptrig.tile([D, SH], F32, name="cosT")
                    sin_t = ptrig.tile([D, SH], F32, name="sinST")
                    nc.scalar.dma_start(out=cos_t, in_=cosT[:, s0:s0 + SH])
                    nc.scalar.dma_start(out=sin_t, in_=sinST[:, s0:s0 + SH])

                    # q/k projection, transposed output [d, s]
                    with tc.tile_pool(name="pw", bufs=4) as pw, \
                         tc.tile_pool(name="ppa", bufs=8, space="PSUM") as ppa:
                        for h in range(NH):
                            for kind, dst in ((0, qT[h]), (1, kT[h])):
                                ot = kind * NH + h
                                wt = pw.tile([128, NKC, 128], MMDT, name="wt")
                                nc.scalar.dma_start(
                                    out=wt,
                                    in_=wqkvT[:, ot * 128:(ot + 1) * 128]
                                    .rearrange("(kc p) o -> p kc o", p=128)
                                    .bitcast(MMDT))
                                for sb_i in range(SH // SB):
                                    ps_t = ppa.tile([128, SB], F32, name="qkps")
                                    for kc in range(NKC):
                                        nc.tensor.matmul(
                                            ps_t, wt[:, kc, :],
                                            xh[kc][:, sb_i * SB:(sb_i + 1) * SB],
                                            start=(kc == 0), stop=(kc == NKC - 1))
                                    lo = s0 + sb_i * SB
                                    nc.any.tensor_copy(dst[:, lo:lo + SB], ps_t)
                            # RoPE for this head's half, in place (hides
                            # under the remaining projection matmuls)
                            for t in (qT[h], kT[h]):
                                sh_t = psh.tile([128, SH], MMDT, name="shuf")
                                nc.gpsimd.dma_start(out=sh_t[0:64, :],
                                                    in_=t[64:128, s0:s0 + SH])
                                nc.gpsimd.dma_start(out=sh_t[64:128, :],
                                                    in_=t[0:64, s0:s0 + SH])
                                nc.vector.tensor_mul(sh_t, sh_t, sin_t)
                                nc.vector.tensor_mul(t[:, s0:s0 + SH],
                                                     t[:, s0:s0 + SH], cos_t)
                                nc.vector.tensor_add(t[:, s0:s0 + SH],
                                                     t[:, s0:s0 + SH], sh_t)

                    # v projection, natural layout [s, 4 heads x d]
                    with tc.tile_pool(name="pwv", bufs=4) as pwv, \
                         tc.tile_pool(name="pvp", bufs=1, space="PSUM") as pvp:
                        vps = [pvp.tile([128, NH * 128], F32, name=f"vps_{st}")
                               for st in range(8)]
                        for kc in range(NKC):
                            wv = pwv.tile([128, NH * 128], MMDT, name="wv")
                            nc.sync.dma_start(
                                out=wv,
                                in_=wqkvT[kc * 128:(kc + 1) * 128,
                                          2 * NH * 128:].bitcast(MMDT))
                            for st in range(8):
                                nc.tensor.matmul(
                                    vps[st],
                                    xh[kc][:, st * 128:(st + 1) * 128],
                                    wv,
                                    start=(kc == 0), stop=(kc == NKC - 1))
                        for st in range(8):
                            sg = half * 8 + st   # global s-chunk
                            nc.any.tensor_copy(vn[sg // 4][:, sg % 4, :], vps[st])

            # ---- Phase B: attention per head ----
            with tc.tile_pool(name="pwo", bufs=1) as pwo:
              wo = []
              for h in range(NH):
                  wt = pwo.tile([128, HID], MMDT, name=f"wo_{h}")
                  nc.scalar.dma_start(
                      out=wt, in_=woT[h * 128:(h + 1) * 128, :].bitcast(MMDT))
                  wo.append(wt)
              with tc.tile_pool(name="pexp", bufs=4) as pexp, \
                   tc.tile_pool(name="prr", bufs=2) as prr, \
                   tc.tile_pool(name="psc", bufs=2, space="PSUM") as psc, \
                   tc.tile_pool(name="plp", bufs=1, space="PSUM") as plp, \
                   tc.tile_pool(name="pop", bufs=3, space="PSUM") as pop:
                  for h in range(NH):
                      for sib in range(NSB):
                          si0 = sib * SB
                          nch = 4 * (sib + 1)      # sj chunks (128 each)
                          l_ps = plp.tile([1, SB], F32, name="lps")
                          o_ps = pop.tile([128, SB], F32, name="ops")
                          for cp in range(nch // 2):
                              s_ps = psc.tile([128, 2, SB], F32, name="sps")
                              e_t = pexp.tile([128, 2, SB], MMDT, name="exp")
                              los = []
                              for j in range(2):
                                  cj = cp * 2 + j
                                  dg = cj - (nch - 4)   # diagonal offset
                                  lo = dg * 128 if dg > 0 else 0
                                  los.append((cj, dg, lo))
                                  nc.tensor.matmul(
                                      s_ps[:, j, lo:],
                                      kT[h][:, cj * 128:(cj + 1) * 128],
                                      qT[h][:, si0 + lo:si0 + SB],
                                      start=True, stop=True)
                                  if dg >= 0:
                                      nc.vector.tensor_add(
                                          s_ps[:, j, lo:lo + 128],
                                          s_ps[:, j, lo:lo + 128], tri_t)
                              if los[0][2] == 0 and los[1][2] == 0:
                                  nc.scalar.activation(
                                      out=e_t, in_=s_ps,
                                      func=mybir.ActivationFunctionType.Exp,
                                      scale=SCALE)
                              else:
                                  for j in range(2):
                                      lo = los[j][2]
                                      nc.scalar.activation(
                                          out=e_t[:, j, lo:], in_=s_ps[:, j, lo:],
                                          func=mybir.ActivationFunctionType.Exp,
                                          scale=SCALE)
                              for j in range(2):
                                  cj, dg, lo = los[j]
                                  nc.tensor.matmul(
                                      l_ps[:, lo:], ones_t, e_t[:, j, lo:],
                                      start=(cj == 0), stop=(cj == nch - 1))
                                  nc.tensor.matmul(
                                      o_ps[:, lo:],
                                      vn[cj // 4][:, cj % 4,
                                                  h * 128:(h + 1) * 128],
                                      e_t[:, j, lo:],
                                      start=(cj == 0), stop=(cj == nch - 1))
                          # 1/l as exp(-ln(l)) on ACT: both functions live
                          # in one table set, and this is ~2x faster than the
                          # single-lane DVE iterative divide
                          lnl = prr.tile([1, SB], F32, name="lnl")
                          nc.scalar.activation(
                              out=lnl, in_=l_ps,
                              func=mybir.ActivationFunctionType.Ln)
                          recip = prr.tile([1, SB], F32, name="recip")
                          nc.scalar.activation(
                              out=recip, in_=lnl,
                              func=mybir.ActivationFunctionType.Exp,
                              scale=-1.0)
                          rb = prr.tile([128, SB], F32, name="rb")
                          nc.gpsimd.partition_broadcast(rb, recip)
                          nc.vector.tensor_mul(
                              outT[h][:, si0:si0 + SB], o_ps, rb)

              # ---- Phase C: partial o-projection ----
              with tc.tile_pool(name="pfin", bufs=4) as pfin, \
                   tc.tile_pool(name="pfp", bufs=4, space="PSUM") as pfp:
                  for st in range(S // 128):
                      fin = pfin.tile([128, HID], F32, name="fin")
                      for ob in range(HID // SB):
                          fps = pfp.tile([128, SB], F32, name="fps")
                          for h in range(NH):
                              nc.tensor.matmul(
                                  fps,
                                  outT[h][:, st * 128:(st + 1) * 128],
                                  wo[h][:, ob * SB:(ob + 1) * SB],
                                  start=(h == 0), stop=(h == NH - 1))
                          nc.any.tensor_copy(fin[:, ob * SB:(ob + 1) * SB], fps)
                      nc.sync.dma_start(
                          out=out[st * 128:(st + 1) * 128, :], in_=fin)

    # Force exp and ln onto the single `natural_log_exp_and_others` ACT
    # table set: with the default map the table-load pass alternates between
    # the exp-only and ln-only sets (~33 reloads x 2.7us on ScalarE). Blank
    # the single-function sets (positions preserved, so set ids stay valid)
    # so both functions resolve to the combined set -> one load.
    import concourse.bacc as _bacc_mod
    import concourse.hw_specs as _hw_specs
    _orig_tables = _hw_specs.get_activation_tables

    def _patched_tables(arch):
        t = dict(_orig_tables(arch))
        for name in ("exp_and_others", "exp_and_friends", "natural_log"):
            if name in t:
                t[name] = set()
        return t

    _bacc_mod.get_activation_tables = _patched_tables
    try:
        nc.compile()
    finally:
        _bacc_mod.get_activation_tables = _orig_tables
    return nc


_NC_CACHE = None


def _get_nc():
    global _NC_CACHE
    if _NC_CACHE is None:
        _NC_CACHE = _build_nc()
    return _NC_CACHE


def _host_inputs(x, w_qkv, w_o):
    """Per-core input maps (sharding + layout prep on host)."""
    inv_freq = 1.0 / (THETA ** (np.arange(0, D, 2, dtype=np.float64) / D))
    pos = np.arange(S, dtype=np.float64)
    freqs = pos[:, None] * inv_freq[None, :]          # (S, D/2)
    emb = np.concatenate([freqs, freqs], axis=-1)     # (S, D)
    cosT = np.ascontiguousarray(np.cos(emb).T.astype(np.float32))   # (D, S)
    sign = np.concatenate([-np.ones(D // 2), np.ones(D // 2)])
    sinST = np.ascontiguousarray((sign[None, :] * np.sin(emb)).T
                                 .astype(np.float32))               # (D, S)
    # additive causal triangle for a diagonal 128x128 block of scores^T:
    # keep (add 0) when sj_local <= si_local, else -1e30
    p = np.arange(128)[:, None]
    f = np.arange(128)[None, :]
    maskadd = np.where(p <= f, 0.0, -1e30).astype(np.float32)       # (128, 128)

    xTb = [np.ascontiguousarray(x[b].T) for b in range(B)]          # (HID, S)
    in_maps = []
    for c in range(NC):
        b, g = c // 4, c % 4
        rows = slice(g * NH * D, (g + 1) * NH * D)
        wq = w_qkv[0 * HID:1 * HID][rows]
        wk = w_qkv[1 * HID:2 * HID][rows]
        wv = w_qkv[2 * HID:3 * HID][rows]
        wqkvT = np.ascontiguousarray(
            np.concatenate([wq, wk, wv], axis=0).T)                 # (HID, 1536)
        woT = np.ascontiguousarray(w_o[:, rows].T)                  # (512, HID)
        in_maps.append({
            "xT": xTb[b], "wqkvT": wqkvT, "woT": woT,
            "cosT": cosT, "sinST": sinST, "maskadd": maskadd,
        })
    return in_maps


def kernel(x, w_qkv, w_o):
    global LAST_RESULT
    x = np.asarray(x, dtype=np.float32)
    w_qkv = np.asarray(w_qkv, dtype=np.float32)
    w_o = np.asarray(w_o, dtype=np.float32)

    nc = _get_nc()
    in_maps = _host_inputs(x, w_qkv, w_o)
    trace = bool(int(os.environ.get("BASS_KERNEL_TRACE", "0")))
    last_exc = None
    for _attempt in range(3):
        try:
            res = run_bass_kernel_spmd(
                nc, in_maps, core_ids=list(range(NC)),
                trace=trace, trace_cores=list(range(NC)) if trace else None)
            break
        except Exception as e:  # transient NRT device errors: retry
            last_exc = e
    else:
        raise last_exc
    LAST_RESULT = res

    out = np.empty((B, S, HID), dtype=np.float32)
    for b in range(B):
        acc = np.zeros((S, HID), dtype=np.float64)
        for g in range(4):
            acc += res.results[b * 4 + g]["out"]
        out[b] = acc.astype(np.float32)
    return out

